# revision 18
# baseline (speedup 1.0000x reference)
"""Center-contrast triplet loss on 8 Trainium2 NeuronCores — collective-free.

Feature-dim sharding: core m gets the m-th 256-wide feature slice of both
inputs as [DS=256, B=4096] fp16 with batch columns reordered k-major so
every per-class K-sum is a short chain of packed halving adds on the DVE.
fp16 is deliberate: the DVE 2x fast path needs 2-byte SAME-TILE packed
operands (fp8 operands, cross-tile adds and strided reduces all run 1x,
measured), so fp8's halved stream loses more on the adds than it saves.

Streaming schedule (two HWDGE queues, round-robin DMA engines):
  - x2 tile t0 ships as four k-quarter chunks so the first tree add
    starts ~1.5us earlier; t1 as two k-halves. Partial sums land in one
    buffer; all combine adds read two halves of a single tile (2x path).
  - x1 ships as class-block spans that shrink toward the end (q0q1, q2,
    q3), (t0, t1) pair per span landing together; 3-level DVE trees.
  - Per class block q: two accumulating PE matmuls (contraction =
    feature partitions, f32 PSUM) form Gram row-block q; ACT casts it to
    fp16 and ships it immediately. The LAST block is column-split into
    two PSUM tiles whose casts run concurrently on DVE and ACT and whose
    DMAs ride both queues, shortening the tail chain.
  - All Gram matmuls precede the ss ones-matmuls in PE order; GpSimd
    (otherwise idle) assembles and ships the ss row.
  - ss = sum_p s2^2 (ACT squares + PE ones-matmuls) ships as [1, 512];
    pp = sum_p s1*s2 is NOT computed on device — it is exactly diag(G),
    read off the shipped Gram on the host.

No on-device collective (ncfw rendezvous ~75us >> 0.5 MB of data): every
core ships its partial Gram + ss row; the host unshard sums the 8
partials and runs the trivial relu/rowmax/cummax/sum epilogue (values are
64x the true ones since centers are kept as sums-of-8; folded at the end).
"""

import numpy as np

import concourse.bacc as bacc
import concourse.mybir as mybir
import concourse.tile as tile
from concourse.bass_utils import run_bass_kernel_spmd
from concourse.vector_clock import ScopedClock


class LeanTileContext(tile.TileContext):
    """TileContext with a drain-only exit.

    The stock exit emits drain + all-engine EVSEM barrier + semaphore
    clears + second barrier. The runtime re-arms semaphores at NEFF
    load/execute, so for this single-shot kernel a drain (which already
    waits on every engine's clock) is sufficient; verified correct across
    repeated executions of the same NEFF.
    """

    def _drain_and_barrier(self, tick_clock, wait_clock):
        drain_inst = self.nc.sync.drain()
        wait_clock.add_sem_waits(
            drain_inst.ins, ScopedClock({None: tick_clock.global_clock})
        )
        popped = self.nc._tile_sem_poison_stack.pop()
        assert popped is self._sem_poison
        sems = list(self.sems.allocated().values())
        sem_nums = [s.num if hasattr(s, "num") else s for s in sems]
        self.nc._state.prepend_free_semaphores(sem_nums)
        for poison_set in self.nc._tile_sem_poison_stack:
            poison_set.update(sem_nums)


N_CORES = 8
B, D, C, K = 4096, 2048, 512, 8
DS = D // N_CORES          # 256 features per core -> 2 partition tiles
NQ = 4                     # class blocks of 128
QC = C // NQ               # 128 classes per block
F32 = mybir.dt.float32
F16 = mybir.dt.float16
BF16 = mybir.dt.bfloat16

# x1 chunking: class-block spans, big early, small at the stream tail
X1_SPANS = [(0, 2), (2, 3), (3, 4)]


def build_nc():
    nc = bacc.Bacc(
        "TRN2", target_bir_lowering=False, debug=False, num_devices=N_CORES
    )
    # x2t columns: k-major over all classes (k*C + c)
    x2t = nc.dram_tensor("x2t", [DS, B], F16, kind="ExternalInput")
    # x1t columns: per span, k-major within span (k*(nq*QC) + c_span)
    x1t = nc.dram_tensor("x1t", [DS, B], F16, kind="ExternalInput")
    v = nc.dram_tensor("v", [C, C], F16, kind="ExternalOutput")
    ab = nc.dram_tensor("ab", [1, C], F32, kind="ExternalOutput")

    with LeanTileContext(nc) as tc:
        with (
            tc.tile_pool(name="sbuf", bufs=1) as pool,
            tc.tile_pool(name="psum", bufs=1, space="PSUM") as psum,
        ):
            const_f32 = pool.tile([128, 1], F32, name="const_f32")
            nc.vector.memset(const_f32[:], 1.0)
            ones_col = pool.tile([128, 1], BF16, name="ones_col")
            nc.vector.tensor_copy(ones_col[:], const_f32[:])

            # tiny first DMAs warm both HWDGE queues before the big stream
            warm_a = pool.tile([1, 64], F16, name="warm_a")
            nc.sync.dma_start(warm_a[:], x2t[0:1, 0:64])
            warm_b = pool.tile([1, 64], F16, name="warm_b")
            nc.scalar.dma_start(warm_b[:], x1t[0:1, 0:64])

            # x2 t0: four k-quarter chunks [128, 1024] (k01, k23 | k45, k67)
            x2_t0q = []
            for qi in range(4):
                eng = nc.sync if qi % 2 == 0 else nc.scalar
                xt = pool.tile([128, B // 4], F16, name=f"x2_0q{qi}")
                eng.dma_start(
                    xt[:],
                    x2t[0:128, (B // 4) * qi : (B // 4) * (qi + 1)],
                )
                x2_t0q.append(xt)
            # x2 t1: two k-half chunks [128, 2048]
            x2_t1h = []
            for h, eng in ((0, nc.sync), (1, nc.scalar)):
                xt = pool.tile([128, B // 2], F16, name=f"x2_1h{h}")
                eng.dma_start(
                    xt[:],
                    x2t[128:256, (B // 2) * h : (B // 2) * (h + 1)],
                )
                x2_t1h.append(xt)

            # x1 span chunks, (t0, span) on sync / (t1, span) on scalar
            x1_ts = {}
            for si, (q0, q1) in enumerate(X1_SPANS):
                w = K * QC * (q1 - q0)
                for t, eng in ((0, nc.sync), (1, nc.scalar)):
                    xq = pool.tile([128, w], F16, name=f"x1_{t}s{si}")
                    eng.dma_start(
                        xq[:],
                        x1t[128 * t : 128 * (t + 1), K * QC * q0 : K * QC * q1],
                    )
                    x1_ts[t, si] = xq

            g_ps = [
                psum.tile([128, C], F32, name=f"g{q}", tag="gps", bufs=NQ - 1)
                for q in range(NQ - 1)
            ]
            # last block column-split over two PSUM tiles for a short tail
            g3 = [
                psum.tile([128, C // 2], F32, name=f"g3{i}", tag="g3", bufs=2)
                for i in range(2)
            ]
            ss_ps = psum.tile([1, C], F32, name="ss_ps")

            def tree3(src, w, tag):
                """3-level packed halving-add K-sum: [128, w] -> [128, w//8]."""
                r1 = pool.tile([128, w // 2], F16, name=f"r1_{tag}")
                nc.vector.tensor_tensor(
                    r1[:], src[:, : w // 2], src[:, w // 2 :],
                    op=mybir.AluOpType.add,
                )
                r2 = pool.tile([128, w // 4], F16, name=f"r2_{tag}")
                nc.vector.tensor_tensor(
                    r2[:], r1[:, : w // 4], r1[:, w // 4 :],
                    op=mybir.AluOpType.add,
                )
                s = pool.tile([128, w // 8], BF16, name=f"s_{tag}")
                nc.vector.tensor_tensor(
                    s[:], r2[:, : w // 8], r2[:, w // 8 :],
                    op=mybir.AluOpType.add,
                )
                return s

            with nc.allow_low_precision(reason="16-bit centers, f32 accum"):
                s2_t, sq_t = [], []

                # s2 t0: per-quarter stage-1 adds into ONE r1 buffer, then
                # same-tile halving to s2_0
                r1_0 = pool.tile([128, B // 2], F16, name="x2r1_0")
                for qi in range(4):
                    src = x2_t0q[qi]
                    nc.vector.tensor_tensor(
                        r1_0[:, (B // 8) * qi : (B // 8) * (qi + 1)],
                        src[:, : B // 8], src[:, B // 8 :],
                        op=mybir.AluOpType.add,
                    )
                r2_0 = pool.tile([128, B // 4], F16, name="x2r2_0")
                nc.vector.tensor_tensor(
                    r2_0[:], r1_0[:, : B // 4], r1_0[:, B // 4 :],
                    op=mybir.AluOpType.add,
                )
                s2_0 = pool.tile([128, C], BF16, name="s2_0")
                nc.vector.tensor_tensor(
                    s2_0[:], r2_0[:, :C], r2_0[:, C:], op=mybir.AluOpType.add
                )
                s2_t.append(s2_0)

                # s2 t1: per-half stage-1 adds into ONE r1 buffer, same-tile
                r1_1 = pool.tile([128, B // 2], F16, name="x2r1_1")
                for h in range(2):
                    src = x2_t1h[h]
                    nc.vector.tensor_tensor(
                        r1_1[:, (B // 4) * h : (B // 4) * (h + 1)],
                        src[:, : B // 4], src[:, B // 4 :],
                        op=mybir.AluOpType.add,
                    )
                r2_1 = pool.tile([128, B // 4], F16, name="x2r2_1")
                nc.vector.tensor_tensor(
                    r2_1[:], r1_1[:, : B // 4], r1_1[:, B // 4 :],
                    op=mybir.AluOpType.add,
                )
                s2_1 = pool.tile([128, C], BF16, name="s2_1")
                nc.vector.tensor_tensor(
                    s2_1[:], r2_1[:, :C], r2_1[:, C:], op=mybir.AluOpType.add
                )
                s2_t.append(s2_1)

                for t in range(2):
                    sq = pool.tile([128, C], BF16, name=f"sq_{t}")
                    nc.scalar.square(sq[:], s2_t[t][:])
                    sq_t.append(sq)

                def emit_ss():
                    # ss ones-matmuls: after q2's Grams, before q3's, so
                    # the ab chain never gates the tail
                    nc.tensor.matmul(
                        ss_ps[:], lhsT=ones_col[:], rhs=sq_t[0][:],
                        start=True, stop=False,
                    )
                    nc.tensor.matmul(
                        ss_ps[:], lhsT=ones_col[:], rhs=sq_t[1][:],
                        start=False, stop=True,
                    )
                    ab_sb = pool.tile([1, C], F32, name="ab_sb")
                    nc.scalar.copy(ab_sb[:], ss_ps[:])
                    nc.gpsimd.dma_start(ab[:], ab_sb[:])

                last_q = X1_SPANS[-1][1] - 1
                for si, (q0, q1) in enumerate(X1_SPANS):
                    w = K * QC * (q1 - q0)
                    s1_t = [
                        tree3(x1_ts[t, si], w, f"x1_{t}s{si}") for t in range(2)
                    ]
                    for q in range(q0, q1):
                        bs = slice(QC * (q - q0), QC * (q - q0 + 1))
                        if q == last_q:
                            emit_ss()
                            # column-split tail: 2 PSUM tiles, 2 queues,
                            # casts concurrently on DVE and ACT
                            for t in range(2):
                                for i in range(2):
                                    nc.tensor.matmul(
                                        g3[i][:],
                                        lhsT=s1_t[t][:, bs],
                                        rhs=s2_t[t][:, C // 2 * i : C // 2 * (i + 1)],
                                        start=(t == 0), stop=(t == 1),
                                    )
                            for i, ceng, deng in (
                                (0, nc.vector, nc.sync),
                                (1, nc.scalar, nc.scalar),
                            ):
                                v_sb = pool.tile(
                                    [128, C // 2], F16, name=f"v_sb3{i}"
                                )
                                if ceng is nc.vector:
                                    nc.vector.tensor_copy(v_sb[:], g3[i][:])
                                else:
                                    nc.scalar.copy(v_sb[:], g3[i][:])
                                deng.dma_start(
                                    v[
                                        QC * q : QC * (q + 1),
                                        C // 2 * i : C // 2 * (i + 1),
                                    ],
                                    v_sb[:],
                                )
                        else:
                            for t in range(2):
                                nc.tensor.matmul(
                                    g_ps[q][:],
                                    lhsT=s1_t[t][:, bs],
                                    rhs=s2_t[t][:],
                                    start=(t == 0), stop=(t == 1),
                                )
                            v_sb = pool.tile([128, C], F16, name=f"v_sb{q}")
                            nc.scalar.copy(v_sb[:], g_ps[q][:])
                            nc.scalar.dma_start(
                                v[QC * q : QC * (q + 1), :], v_sb[:]
                            )

    nc.finalize()
    return nc


def prepare_in_maps(input1, input2):
    x1 = np.asarray(input1, dtype=np.float32)
    x2 = np.asarray(input2, dtype=np.float32)
    # x2: [D, B] with cols k-major over all classes: col = k*C + c
    x2t = np.ascontiguousarray(
        x2.T.reshape(D, C, K).transpose(0, 2, 1), dtype=np.float16
    ).reshape(D, B)
    # x1: [D, B] span-major, k-major within each span
    xr = x1.T.reshape(D, NQ, QC, K)
    cols = []
    for q0, q1 in X1_SPANS:
        slab = xr[:, q0:q1]                      # [D, nq, QC, K]
        cols.append(slab.transpose(0, 3, 1, 2).reshape(D, -1))
    x1t = np.ascontiguousarray(
        np.concatenate(cols, axis=1), dtype=np.float16
    )
    in_maps = []
    for m in range(N_CORES):
        sl = slice(m * DS, (m + 1) * DS)
        in_maps.append({"x1t": x1t[sl], "x2t": x2t[sl]})
    return in_maps


def postprocess(results):
    g = np.zeros((C, C), dtype=np.float32)
    ss = np.zeros(C, dtype=np.float64)
    for m in range(N_CORES):
        g += np.asarray(results[m]["v"], dtype=np.float32)
        ss += np.asarray(results[m]["ab"], dtype=np.float64).reshape(C)
    pp = np.diag(g).astype(np.float64)           # pp_i = G_ii = s1_i . s2_i
    a_col = 0.5 * ss - pp          # per-row bias
    b_row = 0.5 * ss               # per-col bias
    vfull = g + (a_col[:, None] - b_row[None, :]).astype(np.float32)
    rm = np.maximum(vfull.max(axis=1), 0.0) / 32.0
    return np.float32(np.maximum.accumulate(rm).sum())


_NC_CACHE = None


def kernel(input1, input2, targets1, targets2):
    global _NC_CACHE
    if _NC_CACHE is None:
        _NC_CACHE = build_nc()
    in_maps = prepare_in_maps(input1, input2)
    res = run_bass_kernel_spmd(_NC_CACHE, in_maps, list(range(N_CORES)))
    return postprocess(res.results)
